# revision 1
# baseline (speedup 1.0000x reference)
"""Trainium2 Bass kernel v2: 4-head causal+ragged attention, one sample per core.

bf16 matmuls; ragged masking via host-zeroed K/V columns (no kbias);
exp split between ACT (exact, bf16 out) and DVE (Schraudolph bit-trick,
int32 -> f32r bitcast); per-head 32-col AV slices; bias folded into WP.
"""
import sys
sys.path.insert(0, '/opt/trn_rl_repo')
import numpy as np
import ml_dtypes
import concourse.bacc as bacc
import concourse.mybir as mybir
from concourse.tile import TileContext

F32 = mybir.dt.float32
F32R = mybir.dt.float32r
BF16 = mybir.dt.bfloat16
I32 = mybir.dt.int32
I16 = mybir.dt.int16
EXP = mybir.ActivationFunctionType.Exp
MULT = mybir.AluOpType.mult
ADD = mybir.AluOpType.add

S = 2048
D = 64
H = 4
DH = 16
FILL = 1024
LAG = 6

# Schraudolph exp-from-bits: exp(x) ~= bitcast_f32(int32(A*x + B)).
# A = 2^23/ln2; B tuned below (host_prep calibrates nothing at runtime --
# constant chosen to minimize max relative error, ~+-2.98%).
SCH_A = 12102203.161561485
SCH_B = 1065019592.0


def build_nc(num_cores=8, loop_n=1, nkb=16, n_dve=24, stage="full", unroll=1):
    nc = bacc.Bacc("TRN2", target_bir_lowering=False, debug=False,
                   num_devices=num_cores)
    XBT = nc.dram_tensor("xbt", [D + 1, S], BF16, kind="ExternalInput").ap()
    XKV = nc.dram_tensor("xkv", [D + 1, S], BF16, kind="ExternalInput").ap()
    WALL = nc.dram_tensor("wall", [128, 832], BF16, kind="ExternalInput").ap()
    Y = nc.dram_tensor("y", [S, D], F32, kind="ExternalOutput").ap()

    NQC = S // FILL

    # (kb, h) jobs routed to the DVE+Pool Schraudolph exp path, interleaved
    # with ACT jobs so all three engines stay busy.
    dve_jobs = set()
    elig = 0
    for qc in range(S // FILL):
        for kb in range(min(nkb, (qc + 1) * FILL // 128)):
            for h in range(H):
                if elig % 5 < 2 and len(dve_jobs) < n_dve:
                    dve_jobs.add((qc, kb, h))
                elig += 1

    import contextlib
    with TileContext(nc) as tc, nc.allow_low_precision(
            reason="bf16 matmuls and Schraudolph exp intended; accumulation fp32 in PSUM"):
        # Per-head zero-padded V tiles (padding written once, head slices
        # rewritten every iteration). Pool stays open across the loop.
        stack = contextlib.ExitStack()
        vp = stack.enter_context(tc.sbuf_pool(name="vsh", bufs=1))
        v_sh = [vp.tile([128, S], BF16, name=f"vsh{h}") for h in range(H)]
        r4p = [vp.tile([32, S], BF16, name="r4p0")]
        nc.vector.memset(r4p[0][:], 0.0)
        q3 = vp.tile([32, S], BF16, name="q3")
        k3 = vp.tile([32, S], BF16, name="k3")
        nc.vector.memset(q3[:], 0.0)
        nc.vector.memset(k3[:], 0.0)
        for h in range(H):
            nc.vector.memset(v_sh[h][:], 0.0)
        assert loop_n % unroll == 0 or loop_n == 1
        n_steps = loop_n // unroll if loop_n > 1 else 1
        n_body = unroll if loop_n > 1 else 1
        loop_cm = tc.For_i(0, n_steps, 1) if n_steps > 1 else contextlib.nullcontext()
        with stack, loop_cm, \
             tc.sbuf_pool(name="const", bufs=1) as cp, \
             tc.sbuf_pool(name="sb", bufs=1) as sp, \
             tc.sbuf_pool(name="exb", bufs=12) as ebp, \
             tc.sbuf_pool(name="zz", bufs=2) as zp, \
             tc.sbuf_pool(name="ys", bufs=2) as yp:
          for _body in range(n_body):
              # --- constants: one packed DMA ---
              wall = cp.tile([128, 832], BF16)
              nc.sync.dma_start(out=wall[:], in_=WALL[:])
              wq = wall[0:D, 0:128]
              wk = wall[0:D, 128:256]
              wv = wall[0:D + 1, 256:384]
              tril = wall[:, 384:512]
              trir = wall[:, 512:640]
              wp = wall[:, 640:704]
              eb4 = wall[:, 704:832]

              # --- inputs (chunked DMA so projections can start early) ---
              xbt = sp.tile([D + 1, S], BF16)
              xkv = sp.tile([D + 1, S], BF16)
              for c in range(2):
                  cs = slice(c * FILL, (c + 1) * FILL)
                  nc.sync.dma_start(out=xbt[:, cs], in_=XBT[:, cs])
                  nc.scalar.dma_start(out=xkv[:, cs], in_=XKV[:, cs])

              qT = sp.tile([96, S], BF16)
              kT = sp.tile([96, S], BF16)

              # --- projections (q copies on ACT, k copies on DVE) ---
              COPY = mybir.ActivationFunctionType.Copy
              with tc.psum_pool(name="pg", bufs=3) as pg:
                  for half in range(2):
                      for c in range(2):
                          lo = half * FILL + c * 512
                          cl = slice(lo, lo + 512)
                          xs = xbt[0:D, cl]
                          xk = xbt[0:D, cl]
                          pq = pg.tile([128, 512], F32, tag="g")
                          nc.tensor.matmul(pq[:], wq, xs, start=True, stop=True)
                          pk = pg.tile([128, 512], F32, tag="g")
                          nc.tensor.matmul(pk[:], wk, xk, start=True, stop=True)
                          nc.scalar.activation(qT[:, cl], pq[0:96, :], COPY)
                          nc.vector.tensor_copy(kT[:, cl], pk[0:96, :])
                          nc.scalar.activation(q3[0:DH, cl], pq[96:96 + DH, :],
                                               COPY)
                          nc.vector.tensor_copy(k3[0:DH, cl], pk[96:96 + DH, :])
                  # V: transposed layout via x-block as stationary
                  for g in range(4):
                      if nkb <= 4 * g:
                          continue
                      pv = pg.tile([128, 512], F32, tag="g")
                      for i in range(4):
                          kb = 4 * g + i
                          if kb >= nkb:
                              continue
                          nc.tensor.matmul(
                              pv[:, i * 128:(i + 1) * 128],
                              xkv[:, kb * 128:(kb + 1) * 128], wv,
                              start=True, stop=True)
                      vi = min(4, max(0, nkb - 4 * g))
                      srcv = pv.rearrange("p (k c) -> p k c", c=128)
                      for h in range(H):
                          hs = slice(4 + 32 * h, 4 + 32 * h + DH)
                          dst = v_sh[h][:, g * 512:(g + 1) * 512] \
                              .rearrange("p (k c) -> p k c", c=128)
                          nc.vector.tensor_copy(dst[:, 0:vi, hs],
                                                 srcv[:, 0:vi, hs])
                          nc.vector.tensor_copy(dst[:, 0:vi, h:h + 1],
                                                 srcv[:, 0:vi, 0:1])

              # --- attention: one pctx over all queries, single finalize ---
              with tc.psum_pool(name="pc", bufs=1) as pcq:
                  pctx = pcq.tile([128, S], F32, name="pctx")
                  with tc.psum_pool(name="ps", bufs=2) as ps:
                      pend = []

                      def emit_av(job):
                          kb, h, qlo, ex = job
                          q0 = 128 * kb
                          vs = v_sh[h][:, kb * 128:(kb + 1) * 128]
                          for c in range(FILL // 512):
                              s0 = qlo + c * 512
                              if s0 + 512 <= q0:
                                  continue
                              a0 = max(s0, q0)
                              last_kb = min(nkb - 1, (s0 + 511) // 128)
                              nc.tensor.matmul(
                                  pctx[:, a0:s0 + 512],
                                  vs, ex[:, a0 - qlo:s0 + 512 - qlo],
                                  start=(kb == 0 and h == 0),
                                  stop=(kb == last_kb and h == H - 1),
                                  skip_group_check=True)

                      for qc in range(S // FILL):
                          qlo, qhi = qc * FILL, (qc + 1) * FILL
                          for kb in range(min(nkb, (qhi + 127) // 128)):
                              for h in range(H):
                                  q0 = 128 * kb
                                  n0 = max(q0 - qlo, 0)
                                  st = ps.tile([128, FILL], F32, tag="st")
                                  for c in range(FILL // 512):
                                      s0 = qlo + c * 512
                                      if s0 + 512 <= q0:
                                          continue
                                      diag = s0 <= q0 < s0 + 512
                                      b0 = max(s0, q0) - qlo
                                      if h < 3:
                                          kTs = kT[32 * h:32 * h + 32, q0:q0 + 128]
                                          qTs = qT[32 * h:32 * h + 32,
                                                   qlo + b0:s0 + 512]
                                      else:
                                          kTs = k3[:, q0:q0 + 128]
                                          qTs = q3[:, qlo + b0:s0 + 512]
                                      nc.tensor.matmul(
                                          st[:, b0:(c + 1) * 512], kTs, qTs,
                                          start=True, stop=True)
                                  ex = ebp.tile([128, FILL], BF16, tag="exb")
                                  if stage == "st":
                                      pend.append((kb, h, qlo, ex))
                                      continue
                                  nc.scalar.activation(ex[:, n0:FILL],
                                                       st[:, n0:FILL],
                                                       EXP, bias=0.0, scale=0.25)
                                  if q0 >= qlo:
                                      nc.vector.tensor_mul(
                                          ex[:, n0:n0 + 128],
                                          ex[:, n0:n0 + 128], tril)
                                  pend.append((kb, h, qlo, ex))
                                  if len(pend) > LAG and stage == "full":
                                      emit_av(pend.pop(0))
                      while pend and stage == "full":
                          emit_av(pend.pop(0))

                  # --- single finalize over all queries ---
                  if stage == "full":
                      r4c = r4p[0]
                      ys = yp.tile([128, S // 2], F32, tag="ys")
                      with tc.psum_pool(name="pf", bufs=2) as pf:
                          for t in range(S // 512):
                              lo = t * 512
                              nc.vector.reciprocal(r4c[0:H, lo:lo + 512],
                                                   pctx[0:H, lo:lo + 512])
                              rb = pf.tile([128, 512], F32, tag="rb")
                              nc.tensor.matmul(rb[:], eb4[0:32, :],
                                               r4c[:, lo:lo + 512],
                                               start=True, stop=True)
                              rbs = zp.tile([128, 512], BF16, tag="rbs")
                              nc.vector.tensor_copy(rbs[:], rb[:])
                              z = zp.tile([128, 512], BF16, tag="z")
                              nc.vector.tensor_mul(z[:], pctx[:, lo:lo + 512],
                                                   rbs[:])
                              py = pf.tile([128, 256], F32, tag="py")
                              for tb in range(4):
                                  nc.tensor.matmul(py[:, 64 * tb:64 * tb + 64],
                                                   z[:, 128 * tb:128 * tb + 128],
                                                   wp, start=True, stop=True)
                              nc.vector.tensor_copy(ys[:, 256 * t:256 * t + 256],
                                                    py[:])
                      ydst = Y[:, :].rearrange("(g p) d -> p g d", p=128)
                      nc.sync.dma_start(out=ydst, in_=ys.rearrange(
                          "p (g d) -> p g d", d=D))
    nc.compile()
    return nc


def host_prep(x_b, len_b, W_qkv, W_proj, b_proj):
    bf = ml_dtypes.bfloat16
    xbt = np.zeros((D + 1, S), np.float32)
    xbt[0:D] = x_b.T
    xbt[D] = 1.0
    xkv = xbt.copy()
    xkv[:, len_b:] = 0.0
    wq = np.zeros((D, 128), np.float32)
    wk = np.zeros((D, 128), np.float32)
    wv = np.zeros((D + 1, 128), np.float32)
    wp = np.zeros((128, D), np.float32)
    eb4 = np.zeros((128, 128), np.float32)
    for h in range(H):
        wq[:, 32 * h:32 * h + DH] = W_qkv[DH * h:DH * h + DH, :].T
        wk[:, 32 * h:32 * h + DH] = W_qkv[D + DH * h:D + DH * h + DH, :].T
        wv[0:D, 4 + 32 * h:4 + 32 * h + DH] = \
            W_qkv[2 * D + DH * h:2 * D + DH * h + DH, :].T
        wv[D, h] = 1.0
        wp[4 + 32 * h:4 + 32 * h + DH, :] = W_proj[:, DH * h:DH * h + DH].T
        wp[h, :] = np.asarray(b_proj, np.float32) / H
        eb4[h, h] = 1.0
        eb4[h, 4 + 32 * h:4 + 32 * h + DH] = 1.0
    j = np.arange(128)
    tril = (j[:, None] <= j[None, :]).astype(np.float32).astype(bf)
    trir = (j[None, :] <= j[:, None]).astype(bf)
    wall = np.zeros((128, 832), np.float32)
    wall[0:D, 0:128] = wq
    wall[0:D, 128:256] = wk
    wall[0:D + 1, 256:384] = wv
    wall[:, 384:512] = tril.astype(np.float32)
    wall[:, 512:640] = trir.astype(np.float32)
    wall[:, 640:704] = wp
    wall[:, 704:832] = eb4
    return {"xbt": xbt.astype(bf), "xkv": xkv.astype(bf),
            "wall": wall.astype(bf)}


_RUNNER = None


def _build_runner(nc, n_cores=8):
    import jax
    from jax.sharding import Mesh, PartitionSpec
    from jax.experimental.shard_map import shard_map
    from concourse.bass2jax import (_bass_exec_p, install_neuronx_cc_hook,
                                    partition_id_tensor)
    install_neuronx_cc_hook()
    partition_name = nc.partition_id_tensor.name if nc.partition_id_tensor else None
    in_names, out_names, out_avals, zero_outs = [], [], [], []
    for alloc in nc.m.functions[0].allocations:
        if not isinstance(alloc, mybir.MemoryLocationSet):
            continue
        name = alloc.memorylocations[0].name
        if alloc.kind == "ExternalInput":
            if name != partition_name:
                in_names.append(name)
        elif alloc.kind == "ExternalOutput":
            shape = tuple(alloc.tensor_shape)
            dtype = mybir.dt.np(alloc.dtype)
            out_names.append(name)
            out_avals.append(jax.core.ShapedArray(shape, dtype))
            zero_outs.append(np.zeros(shape, dtype))
    n_params = len(in_names)
    n_outs = len(out_avals)
    all_in_names = list(in_names) + list(out_names)
    if partition_name is not None:
        all_in_names.append(partition_name)
    donate = tuple(range(n_params, n_params + n_outs))

    def _body(*args):
        operands = list(args)
        if partition_name is not None:
            operands.append(partition_id_tensor())
        outs = _bass_exec_p.bind(
            *operands,
            out_avals=tuple(out_avals),
            in_names=tuple(all_in_names),
            out_names=tuple(out_names),
            lowering_input_output_aliases=(),
            sim_require_finite=True,
            sim_require_nnan=True,
            nc=nc,
        )
        return tuple(outs)

    devices = jax.devices()[:n_cores]
    mesh = Mesh(np.asarray(devices), ("core",))
    in_specs = (PartitionSpec("core"),) * (n_params + n_outs)
    out_specs = (PartitionSpec("core"),) * n_outs
    sharded = jax.jit(
        shard_map(_body, mesh=mesh, in_specs=in_specs, out_specs=out_specs,
                  check_rep=False),
        donate_argnums=donate, keep_unused=True)

    def run(in_maps):
        import jax
        per_core = [[np.asarray(m[n]) for n in in_names] for m in in_maps]
        concat_in = [np.concatenate([per_core[c][i] for c in range(n_cores)], axis=0)
                     for i in range(n_params)]
        concat_zeros = [np.zeros((n_cores * z.shape[0], *z.shape[1:]), z.dtype)
                        for z in zero_outs]
        out_arrs = sharded(*concat_in, *concat_zeros)
        jax.block_until_ready(out_arrs)
        return [
            {name: np.asarray(out_arrs[i]).reshape(n_cores, *out_avals[i].shape)[c]
             for i, name in enumerate(out_names)}
            for c in range(n_cores)
        ]
    return run






def _numpy_fallback(x, attn_mask, W_qkv, W_proj, b_proj):
    B, S_, D_ = x.shape
    qkv = x @ W_qkv.T
    qkv = qkv.reshape(B, S_, 3, H, DH).transpose(2, 0, 3, 1, 4)
    q, k, v = qkv[0], qkv[1], qkv[2]
    s = np.einsum('bhqd,bhkd->bhqk', q, k).astype(np.float32) / np.sqrt(DH)
    neg = np.finfo(np.float32).min
    s = np.where(attn_mask, s, neg)
    s = s - s.max(-1, keepdims=True)
    p = np.exp(s)
    p = p / p.sum(-1, keepdims=True)
    ctx = np.einsum('bhqk,bhkd->bhqd', p, v)
    ctx = ctx.transpose(0, 2, 1, 3).reshape(B, S_, D_)
    return (ctx @ W_proj.T + b_proj).astype(np.float32)







def kernel(x, attn_mask, W_qkv, W_proj, b_proj):
    global _RUNNER
    x = np.asarray(x, np.float32)
    attn_mask = np.asarray(attn_mask)
    W_qkv = np.asarray(W_qkv, np.float32)
    W_proj = np.asarray(W_proj, np.float32)
    b_proj = np.asarray(b_proj, np.float32)
    B = x.shape[0]
    m = attn_mask[:, 0]
    lens = m[:, -1, :].sum(-1).astype(np.int64)
    pos = np.arange(S)
    causal = pos[:, None] >= pos[None, :]
    structured = bool((lens >= 1).all()) and all(
        np.array_equal(m[b], causal & (pos[None, :] < lens[b])) for b in range(B))
    if not (structured and B == 8 and x.shape == (8, S, D)):
        return _numpy_fallback(x, attn_mask, W_qkv, W_proj, b_proj)
    nkb = int(-(-int(lens.max()) // 128))
    if _RUNNER is None or _RUNNER[0] != nkb:
        nc = build_nc(num_cores=8, nkb=nkb, n_dve=0)
        _RUNNER = (nkb, _build_runner(nc, 8))
    in_maps = [host_prep(x[b], int(lens[b]), W_qkv, W_proj, b_proj)
               for b in range(B)]
    results = _RUNNER[1](in_maps)
    return np.stack([results[c]["y"] for c in range(8)]).astype(np.float32)

